# revision 36
# baseline (speedup 1.0000x reference)
"""Multi-head causal attention (B=4, T=2048, C=1024, H=16, D=64) on 8 trn2 cores.

Sharding: core c owns batch b = c//2 and heads g*8..g*8+7 where g = c%2
(batch-parallel x head-tensor-parallel). Each core computes its 8 heads'
QKV projections, causal attention, and a partial output projection
(columns of Wp belonging to its heads). Host sums the two head-group
partials per batch and adds the bias.

All device matmuls are bf16 (f32 PSUM accumulate); rel-err budget is
2e-2 so bf16's ~0.4% is plenty, and bf16 needs none of the f32r
pad-to-256 hacks.

Structure: ONE fused pipeline over 4 head-PAIRS. While pair p's
attention runs (ACT-heavy: exp softmax), pair p+1's QKV projection
matmuls fill the PE, and the output projection rides the tail of pair
3. Scores for the two heads of a pair are emitted interleaved with
K=64 row-tiling (head A in PE rows 0-63, head B in rows 64-127, via
base_partition-derived tile_position) so the two matmuls run
concurrently - ~2x score throughput vs serial heads.

Per-core SBUF layouts:
  xs   8x [128, 2048] bf16   x^T k-tiles, resident
  QT/KT   [128, 2048] bf16   pair's head dims on partitions (A=0:63, B=64:127)
  Vsb     [128, 16, 2, 65]   V strips per s-tile/head, col 64 = ones (rowsum)
  strip   [128, 1024] bf16   exp(scores^T) for one (s-tile, head, t-half)
  Y    4x [128, 2048] bf16   normalized head outputs, j-major (proj rhs)

Attention per (pair, t-half): for each s-strip: scores-pair matmuls
into a [128,1024] PSUM -> exp (ACT, scale=1/8 fused; no max-subtraction,
scores stay in ~[-3,3]) -> diag tri-mask (DVE). AV runs chunk-major
(one live [65,512] PSUM accumulator per head; strips persist in SBUF)
with the ones column giving the rowsum for free; each chunk is
normalized (reciprocal + rank-1 PE broadcast of 1/rowsum) as soon as
its last strip lands, straight into the SBUF-resident Y.

PSUM budget: scores 2x[128,1024]=4 banks, AV 2x[65,512]=2, mm pool
(QKV/proj/rank-1) 2x[128,512]=2 -> 8 banks exactly.
"""

import numpy as np
from contextlib import ExitStack

B, T, C, H, D = 4, 2048, 1024, 16, 64
HL = H // 2          # 8 heads per core
NP = HL // 2         # 4 head-pairs per core
N_CORES = 8
P = 128
NK = C // P          # 8 contraction tiles for projections
NS = T // P          # 16 s-tiles (key strips)
CH = 512             # t-chunk width (PSUM bank)
NCH = T // CH        # 4 t-chunks

_nc_cache = None


def build_nc():
    global _nc_cache
    if _nc_cache is not None:
        return _nc_cache
    import concourse.bass as bass  # noqa: F401
    import concourse.tile as tile
    from concourse import bacc, mybir

    f32 = mybir.dt.float32
    f32r = mybir.dt.float32r
    bf16 = mybir.dt.bfloat16
    Exp = mybir.ActivationFunctionType.Exp

    fp8 = mybir.dt.float8e4
    NK8 = NK // 2        # 4 doubled contraction tiles for Q/K

    nc = bacc.Bacc("TRN2", target_bir_lowering=False, debug=False,
                   enable_asserts=True, num_devices=N_CORES)
    xT = nc.dram_tensor("xT", (C, T), bf16, kind="ExternalInput").ap()
    # fp8 weights for the Q/K projections (DoubleRow layout [kk*128+p, s*N+n]
    # with contraction row c = 256*kk + 128*s + p). Weights are x64 on the
    # host (e4m3 has no range at 0.02 scale); compensated in the exp scale.
    # The fp8 x copy is derived on-chip by GPSIMD from the bf16 x tiles.
    wq8 = nc.dram_tensor("wq8", (C // 2, HL * D * 2), fp8, kind="ExternalInput").ap()
    wkv = nc.dram_tensor("wkv", (C, 2 * HL * D), bf16, kind="ExternalInput").ap()
    wps = nc.dram_tensor("wps", (HL * D, C), bf16, kind="ExternalInput").ap()
    tri = nc.dram_tensor("tri", (P, P), bf16, kind="ExternalInput").ap()
    onesb = nc.dram_tensor("onesb", (P, NS * HL), bf16, kind="ExternalInput").ap()
    onesf = nc.dram_tensor("onesf", (P, D), f32, kind="ExternalInput").ap()
    o = nc.dram_tensor("o", (C, T), bf16, kind="ExternalOutput").ap()

    with tile.TileContext(nc) as tc:
        with ExitStack() as ctx:
            ctx.enter_context(nc.allow_low_precision(
                reason="bf16 matmuls/strips; rel-err budget 2e-2"))
            sc_ps = ctx.enter_context(tc.tile_pool(name="sc_ps", bufs=2, space="PSUM"))
            av_ps = ctx.enter_context(tc.tile_pool(name="av_ps", bufs=2, space="PSUM"))
            mm_ps = ctx.enter_context(tc.tile_pool(name="mm_ps", bufs=2, space="PSUM"))

            const_pool = ctx.enter_context(tc.tile_pool(name="const", bufs=1))
            tri_sb = const_pool.tile([P, P], bf16, name="tri_sb", tag="tri_sb")
            nc.sync.dma_start(out=tri_sb, in_=tri)
            ones_sb = const_pool.tile([P, D], f32r, name="ones_sb", tag="ones_sb")
            nc.sync.dma_start(out=ones_sb, in_=onesf.bitcast(f32r))

            # resident inputs. DMA order matters: Q/K chains (fp8) go first,
            # then x bf16 + Wv for the V chains, Wp last; spread across both
            # hardware DGE queues.
            # Slab layouts: k-tiles live in the free dim so one strided DMA
            # loads a whole chunk (or weight set) -> ~1 HWDGE issue each
            # instead of 8 (issue overhead is ~630ns serialized per DMA).
            xpool = ctx.enter_context(tc.tile_pool(name="xpool", bufs=1))
            xs2 = xpool.tile([P, NK, T], bf16, name="xs2", tag="xs2")
            xs = [xs2[:, k, :] for k in range(NK)]
            x8 = [xpool.tile([P, 2, T], fp8, name=f"x8_{k}", tag=f"x8_{k}")
                  for k in range(NK8)]
            wpool = ctx.enter_context(tc.tile_pool(name="wpool", bufs=1))
            wq8_sb = wpool.tile([P, NK8, 2, HL * D], fp8, name="wq8sb", tag="wq8sb")
            Wq8 = [wq8_sb[:, k, :, :] for k in range(NK8)]
            wkv_sb = wpool.tile([P, NK, 2 * HL * D], bf16, name="wkvsb", tag="wkvsb")
            Wk_sb = [wkv_sb[:, k, 0:HL * D] for k in range(NK)]
            Wv_sb = [wkv_sb[:, k, HL * D:2 * HL * D] for k in range(NK)]
            wp_sb = wpool.tile([P, NP, C], bf16, name="wpsb", tag="wpsb")
            Wp_sb = [wp_sb[:, j, :] for j in range(NP)]
            queues = [nc.sync, nc.scalar]

            xTr = xT.rearrange("(k p) t -> p k t", k=NK)
            nc.sync.dma_start(out=wkv_sb,
                              in_=wkv.rearrange("(k p) m -> p k m", k=NK))
            nc.scalar.dma_start(out=xs2[:, :, 0:CH], in_=xTr[:, :, 0:CH])
            nc.scalar.dma_start(out=wq8_sb,
                              in_=wq8.rearrange("(k p) (s m) -> p k s m",
                                                k=NK8, s=2))
            for ch in range(1, NCH):
                queues[ch % 2].dma_start(
                    out=xs2[:, :, ch * CH:(ch + 1) * CH],
                    in_=xTr[:, :, ch * CH:(ch + 1) * CH])
            nc.sync.dma_start(out=wp_sb,
                              in_=wps.rearrange("(j p) c -> p j c", j=NP))

            def cvt8(ch):
                # bf16 -> fp8 DoubleRow x copy, on the otherwise-idle GPSIMD
                for k in range(NK8):
                    for s in range(2):
                        nc.gpsimd.tensor_copy(
                            x8[k][:, s, ch * CH:(ch + 1) * CH],
                            xs[2 * k + s][:, ch * CH:(ch + 1) * CH])

            # attention outputs (proj rhs), j-major: Y[p] rows = pair p dims
            ypool = ctx.enter_context(tc.tile_pool(name="ypool", bufs=1))
            Y = [ypool.tile([P, T], bf16, name=f"y{j}", tag=f"y{j}")
                 for j in range(NP)]
            obpool = ctx.enter_context(tc.tile_pool(name="obpool", bufs=3))

            qkpool = ctx.enter_context(tc.tile_pool(name="qkpool", bufs=2))
            vpool = ctx.enter_context(tc.tile_pool(name="vpool", bufs=1))
            strip_pool = ctx.enter_context(tc.tile_pool(name="strip_pool", bufs=34))
            small = ctx.enter_context(tc.tile_pool(name="small", bufs=2))

            def mm(out, lhsT, rhs, **kw):
                nc.tensor.matmul(out, lhsT=lhsT, rhs=rhs,
                                 skip_group_check=True, **kw)

            # V for ALL heads, computed once: the xs[k] stationary tiles are
            # shared by every head, so N=512 moving Wv amortizes LDWEIGHTS
            # 4x vs per-pair N=128 chains. Col 64 of each strip = ones.
            Vsb = vpool.tile([P, NS, HL, D + 1], bf16, name="vsb", tag="vsb")

            def v_init_ones():
                nc.gpsimd.memset(Vsb[:, :, :, D], 1.0)

            def v_chain(s):
                ps = mm_ps.tile([P, HL * D], f32, name="v_ps", tag="mm")
                for k in range(NK):
                    mm(ps, xs[k][:, s * P:(s + 1) * P], Wv_sb[k],
                       start=(k == 0), stop=(k == NK - 1))
                nc.vector.tensor_copy(
                    Vsb[:, s, :, 0:D],
                    ps.rearrange("p (h d) -> p h d", h=HL))

            # ---------- Q/K for one pair ----------
            # Emission is deferred: hand back a list of thunks (chains) so
            # the caller can interleave them with the prior pair's attention.
            def make_qk(p):
                QT = qkpool.tile([P, T], bf16, name=f"qt{p}", tag="qt")
                KT = qkpool.tile([P, T], bf16, name=f"kt{p}", tag="kt")
                chains = []

                def q_chain(ch):
                    # fp8 DoubleRow: 2 contraction subtiles per matmul
                    ps = mm_ps.tile([P, CH], f32, name="qk_ps", tag="mm")
                    for k in range(NK8):
                        mm(ps, Wq8[k][:, :, p * P:(p + 1) * P],
                           x8[k][:, :, ch * CH:(ch + 1) * CH],
                           start=(k == 0), stop=(k == NK8 - 1),
                           perf_mode=mybir.MatmulPerfMode.DoubleRow)
                    nc.vector.tensor_copy(QT[:, ch * CH:(ch + 1) * CH], ps)

                def k_chain(ch):
                    ps = mm_ps.tile([P, CH], f32, name="qk_ps", tag="mm")
                    for k in range(NK):
                        mm(ps, Wk_sb[k][:, p * P:(p + 1) * P],
                           xs[k][:, ch * CH:(ch + 1) * CH],
                           start=(k == 0), stop=(k == NK - 1))
                    nc.vector.tensor_copy(KT[:, ch * CH:(ch + 1) * CH], ps)

                for ch in range(NCH):
                    chains.append(lambda ch=ch: q_chain(ch))
                    chains.append(lambda ch=ch: k_chain(ch))
                return QT, KT, chains

            # ---------- attention for one (pair, half) ----------
            def emit_att(p, half, QT, KT, filler):
                tlo = half * 1024
                ns = 8 if half == 0 else NS
                strips = [[None, None] for _ in range(ns)]
                fill_iter = iter(filler)

                def fill():
                    u = next(fill_iter, None)
                    if u is not None:
                        u()

                def do_scores(i, g):
                    # head g of the pair: PE rows 64g..64g+63 (row-tiled;
                    # the pair's two streams run concurrently on the array)
                    t0 = P * i
                    s0 = max(t0, tlo)
                    off = D * g
                    strip = strip_pool.tile([P, 1024], bf16,
                                            name="strip", tag="strip")
                    strips[i][g] = strip
                    ps = sc_ps.tile([P, 1024], f32, name="sc_ps", tag="sc")
                    b0 = s0
                    while b0 < tlo + 1024:
                        b1 = min((b0 // CH + 1) * CH, tlo + 1024)
                        mm(ps[:, b0 - tlo:b1 - tlo],
                           KT[off:off + D, t0:t0 + P],
                           QT[off:off + D, b0:b1],
                           start=True, stop=True)
                        b0 = b1
                    # scores carry the host-side x64 Q and K weight scaling:
                    # undo 64*64 here along with the 1/sqrt(D) softmax scale
                    nc.scalar.activation(
                        strip[:, s0 - tlo:1024],
                        ps[:, s0 - tlo:1024],
                        Exp, scale=float(1.0 / (np.sqrt(D) * 64.0)))
                    if t0 >= tlo:  # mask the diagonal block
                        nc.vector.tensor_mul(
                            strip[:, t0 - tlo:t0 + P - tlo],
                            strip[:, t0 - tlo:t0 + P - tlo],
                            tri_sb)

                def make_av_chunk(j, g):
                    # AV accumulator for t-chunk j, head g (col 64 = rowsum)
                    av = av_ps.tile([D + 1, CH], f32, name="av", tag="av")

                    def av_mm(i):
                        ts0 = max(CH * j, P * i)
                        mm(av[:, ts0 - CH * j:CH],
                           Vsb[:, i, 2 * p + g, :],
                           strips[i][g][:, ts0 - tlo:CH * (j + 1) - tlo],
                           start=(i == 0), stop=(i == 4 * j + 3))

                    def norm():
                        # reciprocal of the rowsum row (partition 64), then a
                        # rank-1 PE outer product broadcasts it over the D rows
                        rec = small.tile([D + 1, CH], f32r, name="rec", tag="rec")
                        nc.vector.reciprocal(rec[D:D + 1, :], av[D:D + 1, :])
                        rps = mm_ps.tile([D, CH], f32, name="rps", tag="mm")
                        nc.tensor.matmul(rps, lhsT=ones_sb[D:D + 1, 0:D],
                                         rhs=rec[D:D + 1, :],
                                         start=True, stop=True,
                                         skip_group_check=True)
                        rsb = small.tile([D, CH], f32, name="rsb", tag="rsb")
                        nc.vector.tensor_copy(rsb, rps)
                        nc.vector.tensor_mul(
                            Y[p][D * g:D * (g + 1), CH * j:CH * (j + 1)],
                            av[0:D, :], rsb)
                    return av_mm, norm

                jlo, jhi = 2 * half, 2 * half + 1
                nlo = 4 * jlo + 4          # strips feeding chunk jlo
                avlo = [make_av_chunk(jlo, g) for g in range(2)]
                avhi = [make_av_chunk(jhi, g) for g in range(2)]

                for i in range(ns):
                    do_scores(i, 0)
                    do_scores(i, 1)
                    fill()
                    il = i - 2             # avlo lags scores for pipelining
                    if 0 <= il < nlo:
                        avlo[0][0](il)
                        avlo[1][0](il)
                        if il == nlo - 1:
                            avlo[0][1]()
                            avlo[1][1]()
                            fill()
                for il in range(max(0, ns - 2), nlo):   # avlo tail
                    avlo[0][0](il)
                    avlo[1][0](il)
                    if il == nlo - 1:
                        avlo[0][1]()
                        avlo[1][1]()
                for i in range(ns):        # chunk jhi: all strips available
                    avhi[0][0](i)
                    avhi[1][0](i)
                    if i % 4 == 3:
                        fill()
                avhi[0][1]()
                avhi[1][1]()
                for u in fill_iter:        # drain remaining filler
                    u()

            # ---------- output projection ----------
            # Unit = (c-tile, chunk-pair): two matmul chains + evacs share one
            # [128, 1024] staging tile and a single o DMA (HWDGE issue is
            # ~630ns serialized, so halving the DMA count matters at the tail)
            def proj_unit(ct, cp):
                ob = obpool.tile([P, 2 * CH], bf16, name="ob", tag="ob")
                for ch in (2 * cp, 2 * cp + 1):
                    ps = mm_ps.tile([P, CH], f32, name="p_ps", tag="mm")
                    for j in range(NP):
                        mm(ps, Wp_sb[j][:, ct * P:(ct + 1) * P],
                           Y[j][:, ch * CH:(ch + 1) * CH],
                           start=(j == 0), stop=(j == NP - 1))
                    if ch % 2 == 0:
                        nc.vector.tensor_copy(ob[:, 0:CH], ps)
                    else:
                        nc.scalar.copy(ob[:, CH:2 * CH], ps)
                queues[ct % 2].dma_start(
                    out=o[ct * P:(ct + 1) * P, 2 * cp * CH:(2 * cp + 2) * CH],
                    in_=ob)

            # ---------- fused pipeline over pairs ----------
            # Prologue: fp8 x conversion + pair-0 Q/K + V strips 0-7 (enough
            # for half 0), chunk-major to chase the x DMAs as they land.
            # V strips 8-15 ride pair-0 half 0's fill slots.
            v_init_ones()
            qk = make_qk(0)
            for ch in range(NCH):
                cvt8(ch)
                qk[2][2 * ch]()
                qk[2][2 * ch + 1]()
                v_chain(2 * ch)
                v_chain(2 * ch + 1)
            for p in range(NP):
                if p == 0:
                    f0 = [lambda s=s: v_chain(s) for s in range(8, NS)]
                    nxt = make_qk(1)
                    f1 = list(nxt[2])
                elif p < NP - 1:
                    nxt = make_qk(p + 1)
                    chains = list(nxt[2])
                    f0, f1 = chains[:3], chains[3:]
                else:
                    nxt = None
                    # proj chunk-pair 0 needs pair-3 half-0 norms (emitted in
                    # half 0), so it rides half 1; chunk-pair 1 is the tail.
                    f0 = []
                    f1 = [lambda ct=ct: proj_unit(ct, 0)
                          for ct in range(C // P)]
                emit_att(p, 0, qk[0], qk[1], f0)
                emit_att(p, 1, qk[0], qk[1], f1)
                if nxt is not None:
                    qk = nxt
            for ct in range(C // P):
                proj_unit(ct, 1)

    nc.compile()
    _nc_cache = nc
    return nc


def _dr8(a, fp8):
    """[C, M] -> DoubleRow fp8 layout [(k p), (s m)] with c = 256k+128s+p."""
    Cdim, M = a.shape
    a4 = a.reshape(Cdim // 256, 2, 128, M).transpose(0, 2, 1, 3)
    return np.ascontiguousarray(a4.reshape(Cdim // 2, 2 * M)).astype(fp8)


def make_in_maps(x, Wq, Wk, Wv, Wp):
    """Shard FULL inputs into per-core input maps (device layouts)."""
    import ml_dtypes
    bf = ml_dtypes.bfloat16
    fp8 = ml_dtypes.float8_e4m3
    tri = np.triu(np.ones((P, P), dtype=np.float32)).astype(bf)
    in_maps = []
    for c in range(N_CORES):
        b, g = c // 2, c % 2
        hs = slice(g * HL, (g + 1) * HL)
        xb = np.ascontiguousarray(x[b].T)                      # [C, T]
        wqt = Wq[hs].transpose(1, 0, 2).reshape(C, HL * D)     # [C, 512]
        wkt = Wk[hs].transpose(1, 0, 2).reshape(C, HL * D)
        m = {
            "xT": xb.astype(bf),
            "wq8": _dr8(wqt * 64.0, fp8),
            "wkv": np.ascontiguousarray(np.concatenate(
                [wkt, Wv[hs].transpose(1, 0, 2).reshape(C, HL * D)],
                axis=1)).astype(bf),
            "wps": np.ascontiguousarray(
                Wp[:, g * HL * D:(g + 1) * HL * D].T).astype(bf),
            "tri": tri,
            "onesb": np.ones((P, NS * HL), dtype=bf),
            "onesf": np.ones((P, D), dtype=np.float32),
        }
        in_maps.append(m)
    return in_maps


def assemble(results, bp):
    """Sum head-group partials per batch, add bias, transpose back."""
    out = np.empty((B, T, C), dtype=np.float32)
    for b in range(B):
        acc = (results[2 * b]["o"].astype(np.float32)
               + results[2 * b + 1]["o"].astype(np.float32))  # [C, T]
        out[b] = acc.T + bp[None, :]
    return out


def kernel(x, Wq, Wk, Wv, Wp, bp):
    from concourse import bass_utils
    x = np.asarray(x, dtype=np.float32)
    nc = build_nc()
    in_maps = make_in_maps(np.asarray(x), np.asarray(Wq), np.asarray(Wk),
                           np.asarray(Wv), np.asarray(Wp))
    res = bass_utils.run_bass_kernel_spmd(nc, in_maps, core_ids=list(range(N_CORES)))
    return assemble(res.results, np.asarray(bp))


# revision 37
# speedup vs baseline: 1.0606x; 1.0606x over previous
"""Multi-head causal attention (B=4, T=2048, C=1024, H=16, D=64) on 8 trn2 cores.

Sharding: core c owns batch b = c//2 and heads g*8..g*8+7 where g = c%2
(batch-parallel x head-tensor-parallel). Each core computes its 8 heads'
QKV projections, causal attention, and a partial output projection
(columns of Wp belonging to its heads). Host sums the two head-group
partials per batch and adds the bias.

All device matmuls are bf16 (f32 PSUM accumulate); rel-err budget is
2e-2 so bf16's ~0.4% is plenty, and bf16 needs none of the f32r
pad-to-256 hacks.

Structure: ONE fused pipeline over 4 head-PAIRS. While pair p's
attention runs (ACT-heavy: exp softmax), pair p+1's QKV projection
matmuls fill the PE, and the output projection rides the tail of pair
3. Scores for the two heads of a pair are emitted interleaved with
K=64 row-tiling (head A in PE rows 0-63, head B in rows 64-127, via
base_partition-derived tile_position) so the two matmuls run
concurrently - ~2x score throughput vs serial heads.

Per-core SBUF layouts:
  xs   8x [128, 2048] bf16   x^T k-tiles, resident
  QT/KT   [128, 2048] bf16   pair's head dims on partitions (A=0:63, B=64:127)
  Vsb     [128, 16, 2, 65]   V strips per s-tile/head, col 64 = ones (rowsum)
  strip   [128, 1024] bf16   exp(scores^T) for one (s-tile, head, t-half)
  Y    4x [128, 2048] bf16   normalized head outputs, j-major (proj rhs)

Attention per (pair, t-half): for each s-strip: scores-pair matmuls
into a [128,1024] PSUM -> exp (ACT, scale=1/8 fused; no max-subtraction,
scores stay in ~[-3,3]) -> diag tri-mask (DVE). AV runs chunk-major
(one live [65,512] PSUM accumulator per head; strips persist in SBUF)
with the ones column giving the rowsum for free; each chunk is
normalized (reciprocal + rank-1 PE broadcast of 1/rowsum) as soon as
its last strip lands, straight into the SBUF-resident Y.

PSUM budget: scores 2x[128,1024]=4 banks, AV 2x[65,512]=2, mm pool
(QKV/proj/rank-1) 2x[128,512]=2 -> 8 banks exactly.
"""

import numpy as np
from contextlib import ExitStack

B, T, C, H, D = 4, 2048, 1024, 16, 64
HL = H // 2          # 8 heads per core
NP = HL // 2         # 4 head-pairs per core
N_CORES = 8
P = 128
NK = C // P          # 8 contraction tiles for projections
NS = T // P          # 16 s-tiles (key strips)
CH = 512             # t-chunk width (PSUM bank)
NCH = T // CH        # 4 t-chunks

_nc_cache = None


def build_nc():
    global _nc_cache
    if _nc_cache is not None:
        return _nc_cache
    import concourse.bass as bass  # noqa: F401
    import concourse.tile as tile
    from concourse import bacc, mybir

    f32 = mybir.dt.float32
    f32r = mybir.dt.float32r
    bf16 = mybir.dt.bfloat16
    Exp = mybir.ActivationFunctionType.Exp

    fp8 = mybir.dt.float8e4
    NK8 = NK // 2        # 4 doubled contraction tiles for Q/K

    nc = bacc.Bacc("TRN2", target_bir_lowering=False, debug=False,
                   enable_asserts=True, num_devices=N_CORES)
    xT = nc.dram_tensor("xT", (C, T), bf16, kind="ExternalInput").ap()
    # fp8 weights for the Q/K projections (DoubleRow layout [kk*128+p, s*N+n]
    # with contraction row c = 256*kk + 128*s + p). Weights are x64 on the
    # host (e4m3 has no range at 0.02 scale); compensated in the exp scale.
    # The fp8 x copy is derived on-chip by GPSIMD from the bf16 x tiles.
    wq8 = nc.dram_tensor("wq8", (C // 2, HL * D * 2), fp8, kind="ExternalInput").ap()
    wkv = nc.dram_tensor("wkv", (C, 2 * HL * D), bf16, kind="ExternalInput").ap()
    wps = nc.dram_tensor("wps", (HL * D, C), bf16, kind="ExternalInput").ap()
    tri = nc.dram_tensor("tri", (P, P), bf16, kind="ExternalInput").ap()
    o = nc.dram_tensor("o", (C, T), bf16, kind="ExternalOutput").ap()

    with tile.TileContext(nc) as tc:
        with ExitStack() as ctx:
            ctx.enter_context(nc.allow_low_precision(
                reason="bf16 matmuls/strips; rel-err budget 2e-2"))
            sc_ps = ctx.enter_context(tc.tile_pool(name="sc_ps", bufs=2, space="PSUM"))
            av_ps = ctx.enter_context(tc.tile_pool(name="av_ps", bufs=2, space="PSUM"))
            mm_ps = ctx.enter_context(tc.tile_pool(name="mm_ps", bufs=2, space="PSUM"))

            const_pool = ctx.enter_context(tc.tile_pool(name="const", bufs=1))
            tri_sb = const_pool.tile([P, P], bf16, name="tri_sb", tag="tri_sb")
            nc.sync.dma_start(out=tri_sb, in_=tri)

            # resident inputs. DMA order matters: Q/K chains (fp8) go first,
            # then x bf16 + Wv for the V chains, Wp last; spread across both
            # hardware DGE queues.
            # Slab layouts: k-tiles live in the free dim so one strided DMA
            # loads a whole chunk (or weight set) -> ~1 HWDGE issue each
            # instead of 8 (issue overhead is ~630ns serialized per DMA).
            xpool = ctx.enter_context(tc.tile_pool(name="xpool", bufs=1))
            xs2 = xpool.tile([P, NK, T], bf16, name="xs2", tag="xs2")
            xs = [xs2[:, k, :] for k in range(NK)]
            x8 = [xpool.tile([P, 2, T], fp8, name=f"x8_{k}", tag=f"x8_{k}")
                  for k in range(NK8)]
            wpool = ctx.enter_context(tc.tile_pool(name="wpool", bufs=1))
            wq8_sb = wpool.tile([P, NK8, 2, HL * D], fp8, name="wq8sb", tag="wq8sb")
            Wq8 = [wq8_sb[:, k, :, :] for k in range(NK8)]
            wkv_sb = wpool.tile([P, NK, 2 * HL * D], bf16, name="wkvsb", tag="wkvsb")
            Wk_sb = [wkv_sb[:, k, 0:HL * D] for k in range(NK)]
            Wv_sb = [wkv_sb[:, k, HL * D:2 * HL * D] for k in range(NK)]
            wp_sb = wpool.tile([P, NP, C], bf16, name="wpsb", tag="wpsb")
            Wp_sb = [wp_sb[:, j, :] for j in range(NP)]
            queues = [nc.sync, nc.scalar]

            xTr = xT.rearrange("(k p) t -> p k t", k=NK)
            wkvr = wkv.rearrange("(k p) m -> p k m", k=NK)
            nc.sync.dma_start(out=wkv_sb[:, :, 0:HL * D],
                              in_=wkvr[:, :, 0:HL * D])
            nc.scalar.dma_start(out=xs2[:, :, 0:CH], in_=xTr[:, :, 0:CH])
            nc.sync.dma_start(out=wkv_sb[:, :, HL * D:2 * HL * D],
                              in_=wkvr[:, :, HL * D:2 * HL * D])
            nc.scalar.dma_start(out=wq8_sb,
                              in_=wq8.rearrange("(k p) (s m) -> p k s m",
                                                k=NK8, s=2))
            for ch in range(1, NCH):
                queues[ch % 2].dma_start(
                    out=xs2[:, :, ch * CH:(ch + 1) * CH],
                    in_=xTr[:, :, ch * CH:(ch + 1) * CH])
            nc.sync.dma_start(out=wp_sb,
                              in_=wps.rearrange("(j p) c -> p j c", j=NP))

            def cvt8(ch):
                # bf16 -> fp8 DoubleRow x copy, on the otherwise-idle GPSIMD
                for k in range(NK8):
                    for s in range(2):
                        nc.gpsimd.tensor_copy(
                            x8[k][:, s, ch * CH:(ch + 1) * CH],
                            xs[2 * k + s][:, ch * CH:(ch + 1) * CH])

            # attention outputs (proj rhs), j-major: Y[p] rows = pair p dims
            ypool = ctx.enter_context(tc.tile_pool(name="ypool", bufs=1))
            Y = [ypool.tile([P, T], bf16, name=f"y{j}", tag=f"y{j}")
                 for j in range(NP)]
            obpool = ctx.enter_context(tc.tile_pool(name="obpool", bufs=3))

            qkpool = ctx.enter_context(tc.tile_pool(name="qkpool", bufs=2))
            vpool = ctx.enter_context(tc.tile_pool(name="vpool", bufs=1))
            strip_pool = ctx.enter_context(tc.tile_pool(name="strip_pool", bufs=34))
            small = ctx.enter_context(tc.tile_pool(name="small", bufs=2))

            def mm(out, lhsT, rhs, **kw):
                nc.tensor.matmul(out, lhsT=lhsT, rhs=rhs,
                                 skip_group_check=True, **kw)

            # V for ALL heads, computed once: the xs[k] stationary tiles are
            # shared by every head, so N=512 moving Wv amortizes LDWEIGHTS
            # 4x vs per-pair N=128 chains. Col 64 of each strip = ones.
            Vsb = vpool.tile([P, NS, HL, D + 1], bf16, name="vsb", tag="vsb")

            def v_init_ones():
                nc.gpsimd.memset(Vsb[:, :, :, D], 1.0)

            def v_chain(s):
                ps = mm_ps.tile([P, HL * D], f32, name="v_ps", tag="mm")
                for k in range(NK):
                    mm(ps, xs[k][:, s * P:(s + 1) * P], Wv_sb[k],
                       start=(k == 0), stop=(k == NK - 1))
                nc.vector.tensor_copy(
                    Vsb[:, s, :, 0:D],
                    ps.rearrange("p (h d) -> p h d", h=HL))

            # ---------- Q/K for one pair ----------
            # Emission is deferred: hand back a list of thunks (chains) so
            # the caller can interleave them with the prior pair's attention.
            def make_qk(p):
                QT = qkpool.tile([P, T], bf16, name=f"qt{p}", tag="qt")
                KT = qkpool.tile([P, T], bf16, name=f"kt{p}", tag="kt")
                chains = []

                def q_chain(ch):
                    # fp8 DoubleRow: 2 contraction subtiles per matmul
                    ps = mm_ps.tile([P, CH], f32, name="qk_ps", tag="mm")
                    for k in range(NK8):
                        mm(ps, Wq8[k][:, :, p * P:(p + 1) * P],
                           x8[k][:, :, ch * CH:(ch + 1) * CH],
                           start=(k == 0), stop=(k == NK8 - 1),
                           perf_mode=mybir.MatmulPerfMode.DoubleRow)
                    nc.vector.tensor_copy(QT[:, ch * CH:(ch + 1) * CH], ps)

                def k_chain(ch):
                    ps = mm_ps.tile([P, CH], f32, name="qk_ps", tag="mm")
                    for k in range(NK):
                        mm(ps, Wk_sb[k][:, p * P:(p + 1) * P],
                           xs[k][:, ch * CH:(ch + 1) * CH],
                           start=(k == 0), stop=(k == NK - 1))
                    nc.vector.tensor_copy(KT[:, ch * CH:(ch + 1) * CH], ps)

                for ch in range(NCH):
                    chains.append(lambda ch=ch: q_chain(ch))
                    chains.append(lambda ch=ch: k_chain(ch))
                return QT, KT, chains

            # ---------- attention for one (pair, half) ----------
            def emit_att(p, half, QT, KT, filler):
                tlo = half * 1024
                ns = 8 if half == 0 else NS
                strips = [[None, None] for _ in range(ns)]
                fill_iter = iter(filler)

                def fill():
                    u = next(fill_iter, None)
                    if u is not None:
                        u()

                def do_scores(i, g):
                    # head g of the pair: PE rows 64g..64g+63 (row-tiled;
                    # the pair's two streams run concurrently on the array)
                    t0 = P * i
                    s0 = max(t0, tlo)
                    off = D * g
                    strip = strip_pool.tile([P, 1024], bf16,
                                            name="strip", tag="strip")
                    strips[i][g] = strip
                    ps = sc_ps.tile([P, 1024], f32, name="sc_ps", tag="sc")
                    b0 = s0
                    while b0 < tlo + 1024:
                        b1 = min((b0 // CH + 1) * CH, tlo + 1024)
                        mm(ps[:, b0 - tlo:b1 - tlo],
                           KT[off:off + D, t0:t0 + P],
                           QT[off:off + D, b0:b1],
                           start=True, stop=True)
                        b0 = b1
                    # scores carry the host-side x64 Q and K weight scaling:
                    # undo 64*64 here along with the 1/sqrt(D) softmax scale
                    nc.scalar.activation(
                        strip[:, s0 - tlo:1024],
                        ps[:, s0 - tlo:1024],
                        Exp, scale=float(1.0 / (np.sqrt(D) * 64.0)))
                    if t0 >= tlo:  # mask the diagonal block
                        nc.vector.tensor_mul(
                            strip[:, t0 - tlo:t0 + P - tlo],
                            strip[:, t0 - tlo:t0 + P - tlo],
                            tri_sb)

                def make_av_chunk(j, g):
                    # AV accumulator for t-chunk j, head g (col 64 = rowsum)
                    av = av_ps.tile([D + 1, CH], f32, name="av", tag="av")

                    def av_mm(i):
                        ts0 = max(CH * j, P * i)
                        mm(av[:, ts0 - CH * j:CH],
                           Vsb[:, i, 2 * p + g, :],
                           strips[i][g][:, ts0 - tlo:CH * (j + 1) - tlo],
                           start=(i == 0), stop=(i == 4 * j + 3))

                    def norm():
                        # reciprocal of the rowsum row (partition 64), then
                        # GPSIMD broadcasts it over the D rows (PE and DVE
                        # stay out of it entirely)
                        rec = small.tile([D + 1, CH], f32, name="rec", tag="rec")
                        nc.vector.reciprocal(rec[D:D + 1, :], av[D:D + 1, :])
                        rsb = small.tile([D, CH], f32, name="rsb", tag="rsb")
                        nc.gpsimd.partition_broadcast(rsb, rec[D:D + 1, :])
                        nc.vector.tensor_mul(
                            Y[p][D * g:D * (g + 1), CH * j:CH * (j + 1)],
                            av[0:D, :], rsb)
                    return av_mm, norm

                jlo, jhi = 2 * half, 2 * half + 1
                nlo = 4 * jlo + 4          # strips feeding chunk jlo
                avlo = [make_av_chunk(jlo, g) for g in range(2)]
                avhi = [make_av_chunk(jhi, g) for g in range(2)]

                for i in range(ns):
                    do_scores(i, 0)
                    do_scores(i, 1)
                    fill()
                    il = i - 2             # avlo lags scores for pipelining
                    if 0 <= il < nlo:
                        avlo[0][0](il)
                        avlo[1][0](il)
                        if il == nlo - 1:
                            avlo[0][1]()
                            avlo[1][1]()
                            fill()
                for il in range(max(0, ns - 2), nlo):   # avlo tail
                    avlo[0][0](il)
                    avlo[1][0](il)
                    if il == nlo - 1:
                        avlo[0][1]()
                        avlo[1][1]()
                for i in range(ns):        # chunk jhi: all strips available
                    avhi[0][0](i)
                    avhi[1][0](i)
                    if i % 4 == 3:
                        fill()
                avhi[0][1]()
                avhi[1][1]()
                for u in fill_iter:        # drain remaining filler
                    u()

            # ---------- output projection ----------
            # Unit = (c-tile, chunk-pair): two matmul chains + evacs share one
            # [128, 1024] staging tile and a single o DMA (HWDGE issue is
            # ~630ns serialized, so halving the DMA count matters at the tail)
            def proj_unit(ct, cp):
                ob = obpool.tile([P, 2 * CH], bf16, name="ob", tag="ob")
                for ch in (2 * cp, 2 * cp + 1):
                    ps = mm_ps.tile([P, CH], f32, name="p_ps", tag="mm")
                    for j in range(NP):
                        mm(ps, Wp_sb[j][:, ct * P:(ct + 1) * P],
                           Y[j][:, ch * CH:(ch + 1) * CH],
                           start=(j == 0), stop=(j == NP - 1))
                    if ch % 2 == 0:
                        nc.vector.tensor_copy(ob[:, 0:CH], ps)
                    else:
                        nc.scalar.copy(ob[:, CH:2 * CH], ps)
                queues[ct % 2].dma_start(
                    out=o[ct * P:(ct + 1) * P, 2 * cp * CH:(2 * cp + 2) * CH],
                    in_=ob)

            # ---------- fused pipeline over pairs ----------
            # Prologue: fp8 x conversion + pair-0 Q/K + V strips 0-7 (enough
            # for half 0), chunk-major to chase the x DMAs as they land.
            # V strips 8-15 ride pair-0 half 0's fill slots.
            v_init_ones()
            qk = make_qk(0)
            for ch in range(NCH):
                qk[2][2 * ch + 1]()   # K first: no cvt dependency
                cvt8(ch)
                qk[2][2 * ch]()
                v_chain(2 * ch)
                v_chain(2 * ch + 1)
            for p in range(NP):
                if p == 0:
                    f0 = [lambda s=s: v_chain(s) for s in range(8, NS)]
                    nxt = make_qk(1)
                    f1 = list(nxt[2])
                elif p < NP - 1:
                    nxt = make_qk(p + 1)
                    chains = list(nxt[2])
                    f0, f1 = chains[:3], chains[3:]
                else:
                    nxt = None
                    # proj chunk-pair 0 needs pair-3 half-0 norms (emitted in
                    # half 0), so it rides half 1; chunk-pair 1 is the tail.
                    f0 = []
                    f1 = [lambda ct=ct: proj_unit(ct, 0)
                          for ct in range(C // P)]
                emit_att(p, 0, qk[0], qk[1], f0)
                emit_att(p, 1, qk[0], qk[1], f1)
                if nxt is not None:
                    qk = nxt
            for ct in range(C // P):
                proj_unit(ct, 1)

    nc.compile()
    _nc_cache = nc
    return nc


def _dr8(a, fp8):
    """[C, M] -> DoubleRow fp8 layout [(k p), (s m)] with c = 256k+128s+p."""
    Cdim, M = a.shape
    a4 = a.reshape(Cdim // 256, 2, 128, M).transpose(0, 2, 1, 3)
    return np.ascontiguousarray(a4.reshape(Cdim // 2, 2 * M)).astype(fp8)


def make_in_maps(x, Wq, Wk, Wv, Wp):
    """Shard FULL inputs into per-core input maps (device layouts)."""
    import ml_dtypes
    bf = ml_dtypes.bfloat16
    fp8 = ml_dtypes.float8_e4m3
    tri = np.triu(np.ones((P, P), dtype=np.float32)).astype(bf)
    in_maps = []
    for c in range(N_CORES):
        b, g = c // 2, c % 2
        hs = slice(g * HL, (g + 1) * HL)
        xb = np.ascontiguousarray(x[b].T)                      # [C, T]
        wqt = Wq[hs].transpose(1, 0, 2).reshape(C, HL * D)     # [C, 512]
        wkt = Wk[hs].transpose(1, 0, 2).reshape(C, HL * D)
        m = {
            "xT": xb.astype(bf),
            "wq8": _dr8(wqt * 64.0, fp8),
            "wkv": np.ascontiguousarray(np.concatenate(
                [wkt, Wv[hs].transpose(1, 0, 2).reshape(C, HL * D)],
                axis=1)).astype(bf),
            "wps": np.ascontiguousarray(
                Wp[:, g * HL * D:(g + 1) * HL * D].T).astype(bf),
            "tri": tri,
        }
        in_maps.append(m)
    return in_maps


def assemble(results, bp):
    """Sum head-group partials per batch, add bias, transpose back."""
    out = np.empty((B, T, C), dtype=np.float32)
    for b in range(B):
        acc = (results[2 * b]["o"].astype(np.float32)
               + results[2 * b + 1]["o"].astype(np.float32))  # [C, T]
        out[b] = acc.T + bp[None, :]
    return out


def kernel(x, Wq, Wk, Wv, Wp, bp):
    from concourse import bass_utils
    x = np.asarray(x, dtype=np.float32)
    nc = build_nc()
    in_maps = make_in_maps(np.asarray(x), np.asarray(Wq), np.asarray(Wk),
                           np.asarray(Wv), np.asarray(Wp))
    res = bass_utils.run_bass_kernel_spmd(nc, in_maps, core_ids=list(range(N_CORES)))
    return assemble(res.results, np.asarray(bp))


# revision 38
# speedup vs baseline: 1.0653x; 1.0044x over previous
"""Multi-head causal attention (B=4, T=2048, C=1024, H=16, D=64) on 8 trn2 cores.

Sharding: core c owns batch b = c//2 and heads g*8..g*8+7 where g = c%2
(batch-parallel x head-tensor-parallel). Each core computes its 8 heads'
QKV projections, causal attention, and a partial output projection
(columns of Wp belonging to its heads). Host sums the two head-group
partials per batch and adds the bias.

All device matmuls are bf16 (f32 PSUM accumulate); rel-err budget is
2e-2 so bf16's ~0.4% is plenty, and bf16 needs none of the f32r
pad-to-256 hacks.

Structure: ONE fused pipeline over 4 head-PAIRS. While pair p's
attention runs (ACT-heavy: exp softmax), pair p+1's QKV projection
matmuls fill the PE, and the output projection rides the tail of pair
3. Scores for the two heads of a pair are emitted interleaved with
K=64 row-tiling (head A in PE rows 0-63, head B in rows 64-127, via
base_partition-derived tile_position) so the two matmuls run
concurrently - ~2x score throughput vs serial heads.

Per-core SBUF layouts:
  xs   8x [128, 2048] bf16   x^T k-tiles, resident
  QT/KT   [128, 2048] bf16   pair's head dims on partitions (A=0:63, B=64:127)
  Vsb     [128, 16, 2, 65]   V strips per s-tile/head, col 64 = ones (rowsum)
  strip   [128, 1024] bf16   exp(scores^T) for one (s-tile, head, t-half)
  Y    4x [128, 2048] bf16   normalized head outputs, j-major (proj rhs)

Attention per (pair, t-half): for each s-strip: scores-pair matmuls
into a [128,1024] PSUM -> exp (ACT, scale=1/8 fused; no max-subtraction,
scores stay in ~[-3,3]) -> diag tri-mask (DVE). AV runs chunk-major
(one live [65,512] PSUM accumulator per head; strips persist in SBUF)
with the ones column giving the rowsum for free; each chunk is
normalized (reciprocal + rank-1 PE broadcast of 1/rowsum) as soon as
its last strip lands, straight into the SBUF-resident Y.

PSUM budget: scores 2x[128,1024]=4 banks, AV 2x[65,512]=2, mm pool
(QKV/proj/rank-1) 2x[128,512]=2 -> 8 banks exactly.
"""

import numpy as np
from contextlib import ExitStack

B, T, C, H, D = 4, 2048, 1024, 16, 64
HL = H // 2          # 8 heads per core
NP = HL // 2         # 4 head-pairs per core
N_CORES = 8
P = 128
NK = C // P          # 8 contraction tiles for projections
NS = T // P          # 16 s-tiles (key strips)
CH = 512             # t-chunk width (PSUM bank)
NCH = T // CH        # 4 t-chunks

_nc_cache = None


def build_nc():
    global _nc_cache
    if _nc_cache is not None:
        return _nc_cache
    import concourse.bass as bass  # noqa: F401
    import concourse.tile as tile
    from concourse import bacc, mybir

    f32 = mybir.dt.float32
    f32r = mybir.dt.float32r
    bf16 = mybir.dt.bfloat16
    Exp = mybir.ActivationFunctionType.Exp

    fp8 = mybir.dt.float8e4
    NK8 = NK // 2        # 4 doubled contraction tiles for Q/K

    nc = bacc.Bacc("TRN2", target_bir_lowering=False, debug=False,
                   enable_asserts=True, num_devices=N_CORES)
    xT = nc.dram_tensor("xT", (C, T), bf16, kind="ExternalInput").ap()
    # fp8 weights for the Q/K projections (DoubleRow layout [kk*128+p, s*N+n]
    # with contraction row c = 256*kk + 128*s + p). Weights are x64 on the
    # host (e4m3 has no range at 0.02 scale); compensated in the exp scale.
    # The fp8 x copy is derived on-chip by GPSIMD from the bf16 x tiles.
    wq8 = nc.dram_tensor("wq8", (C // 2, HL * D * 2), fp8, kind="ExternalInput").ap()
    wkv = nc.dram_tensor("wkv", (C, 2 * HL * D), bf16, kind="ExternalInput").ap()
    wps = nc.dram_tensor("wps", (HL * D, C), bf16, kind="ExternalInput").ap()
    tri = nc.dram_tensor("tri", (P, P), bf16, kind="ExternalInput").ap()
    o = nc.dram_tensor("o", (C, T), bf16, kind="ExternalOutput").ap()

    with tile.TileContext(nc) as tc:
        with ExitStack() as ctx:
            ctx.enter_context(nc.allow_low_precision(
                reason="bf16 matmuls/strips; rel-err budget 2e-2"))
            sc_ps = ctx.enter_context(tc.tile_pool(name="sc_ps", bufs=2, space="PSUM"))
            av_ps = ctx.enter_context(tc.tile_pool(name="av_ps", bufs=2, space="PSUM"))
            mm_ps = ctx.enter_context(tc.tile_pool(name="mm_ps", bufs=2, space="PSUM"))

            const_pool = ctx.enter_context(tc.tile_pool(name="const", bufs=1))
            tri_sb = const_pool.tile([P, P], bf16, name="tri_sb", tag="tri_sb")
            nc.sync.dma_start(out=tri_sb, in_=tri)

            # resident inputs. DMA order matters: Q/K chains (fp8) go first,
            # then x bf16 + Wv for the V chains, Wp last; spread across both
            # hardware DGE queues.
            # Slab layouts: k-tiles live in the free dim so one strided DMA
            # loads a whole chunk (or weight set) -> ~1 HWDGE issue each
            # instead of 8 (issue overhead is ~630ns serialized per DMA).
            xpool = ctx.enter_context(tc.tile_pool(name="xpool", bufs=1))
            xs2 = xpool.tile([P, NK, T], bf16, name="xs2", tag="xs2")
            xs = [xs2[:, k, :] for k in range(NK)]
            x8 = [xpool.tile([P, 2, T], fp8, name=f"x8_{k}", tag=f"x8_{k}")
                  for k in range(NK8)]
            wpool = ctx.enter_context(tc.tile_pool(name="wpool", bufs=1))
            wq8_sb = wpool.tile([P, NK8, 2, HL * D], fp8, name="wq8sb", tag="wq8sb")
            Wq8 = [wq8_sb[:, k, :, :] for k in range(NK8)]
            wkv_sb = wpool.tile([P, NK, 2 * HL * D], bf16, name="wkvsb", tag="wkvsb")
            Wk_sb = [wkv_sb[:, k, 0:HL * D] for k in range(NK)]
            Wv_sb = [wkv_sb[:, k, HL * D:2 * HL * D] for k in range(NK)]
            wp_sb = wpool.tile([P, NP, C], bf16, name="wpsb", tag="wpsb")
            Wp_sb = [wp_sb[:, j, :] for j in range(NP)]
            queues = [nc.sync, nc.scalar]

            xTr = xT.rearrange("(k p) t -> p k t", k=NK)
            wkvr = wkv.rearrange("(k p) m -> p k m", k=NK)
            nc.sync.dma_start(out=wkv_sb[:, :, 0:HL * D],
                              in_=wkvr[:, :, 0:HL * D])
            nc.scalar.dma_start(out=xs2[:, :, 0:CH], in_=xTr[:, :, 0:CH])
            nc.sync.dma_start(out=wkv_sb[:, :, HL * D:2 * HL * D],
                              in_=wkvr[:, :, HL * D:2 * HL * D])
            nc.scalar.dma_start(out=wq8_sb,
                              in_=wq8.rearrange("(k p) (s m) -> p k s m",
                                                k=NK8, s=2))
            for ch in range(1, NCH):
                queues[ch % 2].dma_start(
                    out=xs2[:, :, ch * CH:(ch + 1) * CH],
                    in_=xTr[:, :, ch * CH:(ch + 1) * CH])
            nc.sync.dma_start(out=wp_sb,
                              in_=wps.rearrange("(j p) c -> p j c", j=NP))

            def cvt8(ch):
                # bf16 -> fp8 DoubleRow x copy, on the otherwise-idle GPSIMD
                for k in range(NK8):
                    for s in range(2):
                        nc.gpsimd.tensor_copy(
                            x8[k][:, s, ch * CH:(ch + 1) * CH],
                            xs[2 * k + s][:, ch * CH:(ch + 1) * CH])

            # attention outputs (proj rhs), j-major: Y[p] rows = pair p dims
            ypool = ctx.enter_context(tc.tile_pool(name="ypool", bufs=1))
            Y = [ypool.tile([P, T], bf16, name=f"y{j}", tag=f"y{j}")
                 for j in range(NP)]
            obpool = ctx.enter_context(tc.tile_pool(name="obpool", bufs=3))

            qkpool = ctx.enter_context(tc.tile_pool(name="qkpool", bufs=2))
            vpool = ctx.enter_context(tc.tile_pool(name="vpool", bufs=1))
            strip_pool = ctx.enter_context(tc.tile_pool(name="strip_pool", bufs=34))
            small = ctx.enter_context(tc.tile_pool(name="small", bufs=2))

            def mm(out, lhsT, rhs, **kw):
                nc.tensor.matmul(out, lhsT=lhsT, rhs=rhs,
                                 skip_group_check=True, **kw)

            # V for ALL heads, computed once: the xs[k] stationary tiles are
            # shared by every head, so N=512 moving Wv amortizes LDWEIGHTS
            # 4x vs per-pair N=128 chains. Col 64 of each strip = ones.
            Vsb = vpool.tile([P, NS, HL, D + 1], bf16, name="vsb", tag="vsb")

            def v_init_ones():
                nc.gpsimd.memset(Vsb[:, :, :, D], 1.0)

            def v_chain(s):
                ps = mm_ps.tile([P, HL * D], f32, name="v_ps", tag="mm")
                for k in range(NK):
                    mm(ps, xs[k][:, s * P:(s + 1) * P], Wv_sb[k],
                       start=(k == 0), stop=(k == NK - 1))
                nc.vector.tensor_copy(
                    Vsb[:, s, :, 0:D],
                    ps.rearrange("p (h d) -> p h d", h=HL))

            # ---------- Q/K for one pair ----------
            # Emission is deferred: hand back a list of thunks (chains) so
            # the caller can interleave them with the prior pair's attention.
            def make_qk(p):
                QT = qkpool.tile([P, T], bf16, name=f"qt{p}", tag="qt")
                KT = qkpool.tile([P, T], bf16, name=f"kt{p}", tag="kt")
                chains = []

                def q_chain(ch):
                    # fp8 DoubleRow: 2 contraction subtiles per matmul
                    ps = mm_ps.tile([P, CH], f32, name="qk_ps", tag="mm")
                    for k in range(NK8):
                        mm(ps, Wq8[k][:, :, p * P:(p + 1) * P],
                           x8[k][:, :, ch * CH:(ch + 1) * CH],
                           start=(k == 0), stop=(k == NK8 - 1),
                           perf_mode=mybir.MatmulPerfMode.DoubleRow)
                    nc.vector.tensor_copy(QT[:, ch * CH:(ch + 1) * CH], ps)

                def k_chain(ch):
                    ps = mm_ps.tile([P, CH], f32, name="qk_ps", tag="mm")
                    for k in range(NK):
                        mm(ps, Wk_sb[k][:, p * P:(p + 1) * P],
                           xs[k][:, ch * CH:(ch + 1) * CH],
                           start=(k == 0), stop=(k == NK - 1))
                    nc.vector.tensor_copy(KT[:, ch * CH:(ch + 1) * CH], ps)

                for ch in range(NCH):
                    chains.append(lambda ch=ch: q_chain(ch))
                    chains.append(lambda ch=ch: k_chain(ch))
                return QT, KT, chains

            # ---------- attention for one (pair, half) ----------
            def emit_att(p, half, QT, KT, filler):
                tlo = half * 1024
                ns = 8 if half == 0 else NS
                strips = [[None, None] for _ in range(ns)]
                fill_iter = iter(filler)

                def fill():
                    u = next(fill_iter, None)
                    if u is not None:
                        u()

                def do_scores(i, g):
                    # head g of the pair: PE rows 64g..64g+63 (row-tiled;
                    # the pair's two streams run concurrently on the array)
                    t0 = P * i
                    s0 = max(t0, tlo)
                    off = D * g
                    strip = strip_pool.tile([P, 1024], bf16,
                                            name="strip", tag="strip")
                    strips[i][g] = strip
                    ps = sc_ps.tile([P, 1024], f32, name="sc_ps", tag="sc")
                    b0 = s0
                    while b0 < tlo + 1024:
                        b1 = min((b0 // CH + 1) * CH, tlo + 1024)
                        mm(ps[:, b0 - tlo:b1 - tlo],
                           KT[off:off + D, t0:t0 + P],
                           QT[off:off + D, b0:b1],
                           start=True, stop=True)
                        b0 = b1
                    # scores carry the host-side x64 Q and K weight scaling:
                    # undo 64*64 here along with the 1/sqrt(D) softmax scale
                    nc.scalar.activation(
                        strip[:, s0 - tlo:1024],
                        ps[:, s0 - tlo:1024],
                        Exp, scale=float(1.0 / (np.sqrt(D) * 64.0)))
                    if t0 >= tlo:  # mask the diagonal block
                        nc.vector.tensor_mul(
                            strip[:, t0 - tlo:t0 + P - tlo],
                            strip[:, t0 - tlo:t0 + P - tlo],
                            tri_sb)

                def make_av_chunk(j, g):
                    # AV accumulator for t-chunk j, head g (col 64 = rowsum)
                    av = av_ps.tile([D + 1, CH], f32, name="av", tag="av")

                    def av_mm(i):
                        ts0 = max(CH * j, P * i)
                        mm(av[:, ts0 - CH * j:CH],
                           Vsb[:, i, 2 * p + g, :],
                           strips[i][g][:, ts0 - tlo:CH * (j + 1) - tlo],
                           start=(i == 0), stop=(i == 4 * j + 3))

                    def norm():
                        # reciprocal of the rowsum row (partition 64), then
                        # GPSIMD broadcasts it over the D rows (PE and DVE
                        # stay out of it entirely)
                        rec = small.tile([D + 1, CH], f32, name="rec", tag="rec")
                        nc.vector.reciprocal(rec[D:D + 1, :], av[D:D + 1, :])
                        rsb = small.tile([D, CH], f32, name="rsb", tag="rsb")
                        nc.gpsimd.partition_broadcast(rsb, rec[D:D + 1, :])
                        nc.vector.tensor_mul(
                            Y[p][D * g:D * (g + 1), CH * j:CH * (j + 1)],
                            av[0:D, :], rsb)
                    return av_mm, norm

                jlo, jhi = 2 * half, 2 * half + 1
                nlo = 4 * jlo + 4          # strips feeding chunk jlo
                avlo = [make_av_chunk(jlo, g) for g in range(2)]
                avhi = [make_av_chunk(jhi, g) for g in range(2)]

                for i in range(ns):
                    do_scores(i, 0)
                    do_scores(i, 1)
                    fill()
                    il = i - 2             # avlo lags scores for pipelining
                    if 0 <= il < nlo:
                        avlo[0][0](il)
                        avlo[1][0](il)
                        if il == nlo - 1:
                            avlo[0][1]()
                            avlo[1][1]()
                            fill()
                for il in range(max(0, ns - 2), nlo):   # avlo tail
                    avlo[0][0](il)
                    avlo[1][0](il)
                    if il == nlo - 1:
                        avlo[0][1]()
                        avlo[1][1]()
                for i in range(ns):        # chunk jhi: all strips available
                    avhi[0][0](i)
                    avhi[1][0](i)
                    if i % 4 == 3:
                        fill()
                avhi[0][1]()
                avhi[1][1]()
                for u in fill_iter:        # drain remaining filler
                    u()

            # ---------- output projection ----------
            # Unit = (c-tile, chunk-pair): two matmul chains + evacs share one
            # [128, 1024] staging tile and a single o DMA (HWDGE issue is
            # ~630ns serialized, so halving the DMA count matters at the tail)
            def proj_unit(ct, cp):
                ob = obpool.tile([P, 2 * CH], bf16, name="ob", tag="ob")
                for ch in (2 * cp, 2 * cp + 1):
                    ps = mm_ps.tile([P, CH], f32, name="p_ps", tag="mm")
                    for j in range(NP):
                        mm(ps, Wp_sb[j][:, ct * P:(ct + 1) * P],
                           Y[j][:, ch * CH:(ch + 1) * CH],
                           start=(j == 0), stop=(j == NP - 1))
                    nc.vector.tensor_copy(
                        ob[:, (ch % 2) * CH:(ch % 2 + 1) * CH], ps)
                queues[ct % 2].dma_start(
                    out=o[ct * P:(ct + 1) * P, 2 * cp * CH:(2 * cp + 2) * CH],
                    in_=ob)

            # ---------- fused pipeline over pairs ----------
            # Prologue: fp8 x conversion + pair-0 Q/K + V strips 0-7 (enough
            # for half 0), chunk-major to chase the x DMAs as they land.
            # V strips 8-15 ride pair-0 half 0's fill slots.
            v_init_ones()
            qk = make_qk(0)
            for ch in range(NCH):
                qk[2][2 * ch + 1]()   # K first: no cvt dependency
                cvt8(ch)
                qk[2][2 * ch]()
                v_chain(2 * ch)
                v_chain(2 * ch + 1)
            for p in range(NP):
                if p == 0:
                    f0 = [lambda s=s: v_chain(s) for s in range(8, NS)]
                    nxt = make_qk(1)
                    f1 = list(nxt[2])
                elif p < NP - 1:
                    nxt = make_qk(p + 1)
                    chains = list(nxt[2])
                    f0, f1 = chains[:3], chains[3:]
                else:
                    nxt = None
                    # proj chunk-pair 0 needs pair-3 half-0 norms (emitted in
                    # half 0), so it rides half 1; chunk-pair 1 is the tail.
                    f0 = []
                    f1 = [lambda ct=ct: proj_unit(ct, 0)
                          for ct in range(C // P)]
                emit_att(p, 0, qk[0], qk[1], f0)
                emit_att(p, 1, qk[0], qk[1], f1)
                if nxt is not None:
                    qk = nxt
            for ct in range(C // P):
                proj_unit(ct, 1)

    nc.compile()
    _nc_cache = nc
    return nc


def _dr8(a, fp8):
    """[C, M] -> DoubleRow fp8 layout [(k p), (s m)] with c = 256k+128s+p."""
    Cdim, M = a.shape
    a4 = a.reshape(Cdim // 256, 2, 128, M).transpose(0, 2, 1, 3)
    return np.ascontiguousarray(a4.reshape(Cdim // 2, 2 * M)).astype(fp8)


def make_in_maps(x, Wq, Wk, Wv, Wp):
    """Shard FULL inputs into per-core input maps (device layouts)."""
    import ml_dtypes
    bf = ml_dtypes.bfloat16
    fp8 = ml_dtypes.float8_e4m3
    tri = np.triu(np.ones((P, P), dtype=np.float32)).astype(bf)
    in_maps = []
    for c in range(N_CORES):
        b, g = c // 2, c % 2
        hs = slice(g * HL, (g + 1) * HL)
        xb = np.ascontiguousarray(x[b].T)                      # [C, T]
        wqt = Wq[hs].transpose(1, 0, 2).reshape(C, HL * D)     # [C, 512]
        wkt = Wk[hs].transpose(1, 0, 2).reshape(C, HL * D)
        m = {
            "xT": xb.astype(bf),
            "wq8": _dr8(wqt * 64.0, fp8),
            "wkv": np.ascontiguousarray(np.concatenate(
                [wkt, Wv[hs].transpose(1, 0, 2).reshape(C, HL * D)],
                axis=1)).astype(bf),
            "wps": np.ascontiguousarray(
                Wp[:, g * HL * D:(g + 1) * HL * D].T).astype(bf),
            "tri": tri,
        }
        in_maps.append(m)
    return in_maps


def assemble(results, bp):
    """Sum head-group partials per batch, add bias, transpose back."""
    out = np.empty((B, T, C), dtype=np.float32)
    for b in range(B):
        acc = (results[2 * b]["o"].astype(np.float32)
               + results[2 * b + 1]["o"].astype(np.float32))  # [C, T]
        out[b] = acc.T + bp[None, :]
    return out


def kernel(x, Wq, Wk, Wv, Wp, bp):
    from concourse import bass_utils
    x = np.asarray(x, dtype=np.float32)
    nc = build_nc()
    in_maps = make_in_maps(np.asarray(x), np.asarray(Wq), np.asarray(Wk),
                           np.asarray(Wv), np.asarray(Wp))
    res = bass_utils.run_bass_kernel_spmd(nc, in_maps, core_ids=list(range(N_CORES)))
    return assemble(res.results, np.asarray(bp))
